# revision 1
# baseline (speedup 1.0000x reference)
"""Cross-attention kernel for Trainium2, 8 NeuronCores.

Problem (full shapes): B=4, Lq=Lk=2048, D(in)=D(out)=1024, fp32.
    q = query @ Wq + bq ; k = key @ Wk + bk ; v = value @ Wv + bv
    out = softmax(q k^T / sqrt(1024)) v

Sharding: 8 cores = (batch b, half h of Lq). Each core computes a
[1024, 1024] slice of the output for batch b, query rows
[h*1024, (h+1)*1024). K/V projections for a batch are duplicated across
the core pair (cheaper than cross-core exchange for this size).

Per-core layouts (P=128 partitions):
  - Projections are computed with the contraction dim (d) on partitions;
    host supplies q/k/v inputs pre-transposed ([D, L]).
  - qT [do, lq] and kT [do, lk] are produced feature-major, v [lk, do]
    natural.  Scores are computed transposed, ST[lk, lq], so the whole
    attention pipeline needs no on-device transposes.
  - Scores are small (|S/32| < ~3) so exp is applied without
    max-subtraction; row sums r[lq] come from a PE matmul with a ones
    column; normalization is a per-partition scalar multiply on the
    natural-layout output.  v carries its bias, so softmax rows summing
    to 1 makes the output bias exact with no extra broadcast add.
"""

import os
import sys

sys.path.insert(0, "/opt/trn_rl_repo")

from contextlib import ExitStack

import numpy as np

import concourse.bass as bass
import concourse.tile as tile
from concourse import bacc, mybir
from concourse.bass_utils import run_bass_kernel_spmd

P = 128
B, LQ, LK, D = 4, 2048, 2048, 1024
NCORES = 8
LQS = LQ * B // NCORES  # 1024 query rows per core
KCH = D // P  # 8 contraction chunks
DOT = D // P  # 8 output-feature tiles
LKT = LK // P  # 16 key tiles
PBLK = 512  # projection matmul free dim
ABLK = 256  # attention lq block (2 lq tiles)
SCALE = 1.0 / 32.0  # 1/sqrt(D)

F32 = mybir.dt.float32

# Matmul dtype mode: "f32" (exact, 4 cyc/row), "f32r" (fp32 storage
# rounded to a ~12-bit mantissa, single-pass matmul at bf16 rate, 1
# cyc/row for free dim >= 256), "bf16".
MM_MODE = os.environ.get("KMODE", "f32r")


def _mm_dtypes(mode):
    if mode == "bf16":
        import ml_dtypes

        return mybir.dt.bfloat16, np.dtype(ml_dtypes.bfloat16)
    if mode == "f32r":
        return mybir.dt.float32r, np.dtype(np.float32)
    return F32, np.dtype(np.float32)


DT, HOST_DT = _mm_dtypes(MM_MODE)


def _round_host(a):
    """Round fp32 host data the way the hardware rounds float32r
    producers (round-to-nearest-even keeping 11 mantissa bits), so
    DMA-fed f32r operands match what on-device rounding would give."""
    if MM_MODE != "f32r":
        return np.ascontiguousarray(a)
    b = np.ascontiguousarray(a).view(np.uint32).astype(np.uint64)
    keep = 12  # bits dropped
    mask = (np.uint64(0xFFFFFFFF) >> np.uint64(keep)) << np.uint64(keep)
    half = np.uint64(1) << np.uint64(keep - 1)
    low = b & ~mask & np.uint64(0xFFFFFFFF)
    rup = (b + half) & mask
    tie = low == half
    lsb = (b >> np.uint64(keep)) & np.uint64(1)
    out = rup
    down = tie & (lsb == 0)
    out[down] = (b & mask)[down]
    return out.astype(np.uint32).view(np.float32)


def build_program(repeat=1):
    nc = bacc.Bacc("TRN2", target_bir_lowering=False, debug=False)

    q_inT = nc.dram_tensor("q_inT", [D, LQS], DT, kind="ExternalInput").ap()
    k_inT = nc.dram_tensor("k_inT", [D, LK], DT, kind="ExternalInput").ap()
    v_inT = nc.dram_tensor("v_inT", [D, LK], DT, kind="ExternalInput").ap()
    Wq = nc.dram_tensor("Wq", [D, D], DT, kind="ExternalInput").ap()
    Wk = nc.dram_tensor("Wk", [D, D], DT, kind="ExternalInput").ap()
    Wv = nc.dram_tensor("Wv", [D, D], DT, kind="ExternalInput").ap()
    bq = nc.dram_tensor("bq", [D], F32, kind="ExternalInput").ap()
    bk = nc.dram_tensor("bk", [D], F32, kind="ExternalInput").ap()
    bv = nc.dram_tensor("bv", [D], F32, kind="ExternalInput").ap()
    out = nc.dram_tensor("out", [LQS, D], F32, kind="ExternalOutput").ap()

    q_inT_t = q_inT.rearrange("(o p) l -> p o l", p=P)
    k_inT_t = k_inT.rearrange("(o p) l -> p o l", p=P)
    v_inT_t = v_inT.rearrange("(o p) l -> p o l", p=P)
    Wq_t = Wq.rearrange("(o p) n -> p o n", p=P)
    Wk_t = Wk.rearrange("(o p) n -> p o n", p=P)
    Wv_t = Wv.rearrange("(o p) n -> p o n", p=P)

    with tile.TileContext(nc) as tc, ExitStack() as ctx:
        psum_mm = ctx.enter_context(tc.tile_pool(name="psum_mm", bufs=4, space="PSUM"))
        psum_st = ctx.enter_context(tc.tile_pool(name="psum_st", bufs=2, space="PSUM"))
        psum_r = ctx.enter_context(tc.tile_pool(name="psum_r", bufs=2, space="PSUM"))
        singles = ctx.enter_context(tc.tile_pool(name="singles", bufs=1))
        kt_pool = ctx.enter_context(tc.tile_pool(name="kt_pool", bufs=1))
        v_pool = ctx.enter_context(tc.tile_pool(name="v_pool", bufs=1))
        dram = ctx.enter_context(tc.tile_pool(name="dram", bufs=1, space="DRAM"))
        tc._pt_pool = ctx.enter_context(tc.tile_pool(name="pt_pool", bufs=6))

        # ---- constants -------------------------------------------------
        bq_sb = singles.tile([P, DOT], F32, name="bq_sb")
        nc.sync.dma_start(bq_sb[:], bq.rearrange("(o p) -> p o", p=P))
        bk_sb = singles.tile([P, DOT], F32, name="bk_sb")
        nc.sync.dma_start(bk_sb[:], bk.rearrange("(o p) -> p o", p=P))
        # bv broadcast to all 128 partitions (stride-0 partition read)
        bv_rep = singles.tile([P, D], F32, name="bv_rep")
        bv_bcast = bass.AP(tensor=bv.tensor, offset=bv.offset, ap=[[0, P], *bv.ap])
        nc.gpsimd.dma_start(bv_rep[:], bv_bcast)
        if MM_MODE == "f32r":
            ones_f = singles.tile([P, 2], F32, name="ones_f")
            nc.vector.memset(ones_f[:], 1.0)
            ones_sb = singles.tile([P, 2], DT, name="ones_sb")
            nc.vector.tensor_scalar_add(ones_sb[:], ones_f[:], 0.0)
        else:
            ones_sb = singles.tile([P, 2], DT, name="ones_sb")
            nc.vector.memset(ones_sb[:], 1.0)

        phases = os.environ.get("KPHASES", "all")
        for _rep in range(repeat):
            one_pass(nc, tc, singles, kt_pool, v_pool, dram, psum_mm, psum_st,
                     psum_r, bq_sb, bk_sb, bv_rep, ones_sb,
                     q_inT_t, k_inT_t, v_inT_t, Wq_t, Wk_t, Wv_t, out,
                     phases=phases)

    nc.compile()
    return nc


def pt_pool_tile(tc, name):
    return tc._pt_pool.tile([P, ABLK], DT, tag="pt", name=name)


def one_pass(nc, tc, singles, kt_pool, v_pool, dram, psum_mm, psum_st, psum_r,
             bq_sb, bk_sb, bv_rep, ones_sb,
             q_inT_t, k_inT_t, v_inT_t, Wq_t, Wk_t, Wv_t, out, phases="all"):
    do_proj = phases in ("all", "proj")
    do_attn = phases in ("all", "attn")
    with tc.tile_pool(name="wpool", bufs=1) as wpool:
        kT_sb = kt_pool.tile([P, DOT, LK], DT, tag="kT_sb", name="kT_sb")
        v_sb = v_pool.tile([P, LKT, D], DT, tag="v_sb", name="v_sb")

        # ---- phase 0: kT = (k_in @ Wk + bk)^T, SBUF-resident ----------
        if not do_proj:
            nc.vector.memset(kT_sb[:, 0, 0:2].bitcast(F32), 0.001)
            nc.vector.memset(v_sb[:, 0, 0:2].bitcast(F32), 0.001)
        if do_proj:
          with tc.tile_pool(name="kin_pool", bufs=2) as kin_pool:
            Wk_sb = wpool.tile([P, KCH, D], DT, tag="W", name="Wk_sb")
            for o in range(KCH):
                nc.sync.dma_start(Wk_sb[:, o], Wk_t[:, o])
            for n in range(LK // PBLK):
                kin = kin_pool.tile([P, KCH, PBLK], DT, tag="kin", name="kin")
                for o in range(KCH):
                    eng = nc.sync if o % 2 == 0 else nc.scalar
                    eng.dma_start(
                        kin[:, o], k_inT_t[:, o, n * PBLK : (n + 1) * PBLK]
                    )
                for m in range(DOT):
                    ps = psum_mm.tile([P, PBLK], F32, tag="mm", name="ps_k")
                    for k in range(KCH):
                        nc.tensor.matmul(
                            ps[:],
                            Wk_sb[:, k, m * P : (m + 1) * P],
                            kin[:, k],
                            start=(k == 0),
                            stop=(k == KCH - 1),
                        )
                    nc.vector.tensor_scalar_add(
                        kT_sb[:, m, n * PBLK : (n + 1) * PBLK],
                        ps[:],
                        bk_sb[:, m : m + 1],
                    )

        # ---- phase 1: v = v_in @ Wv + bv, natural layout, resident ----
        if do_proj:
          with tc.tile_pool(name="vin_pool", bufs=2) as vin_pool:
            Wv_sb = wpool.tile([P, KCH, D], DT, tag="W", name="Wv_sb")
            for o in range(KCH):
                nc.sync.dma_start(Wv_sb[:, o], Wv_t[:, o])
            for blk in range(LK // PBLK):  # 4 blocks of 512 key rows
                vin = vin_pool.tile([P, KCH, PBLK], DT, tag="vin", name="vin")
                for o in range(KCH):
                    eng = nc.sync if o % 2 == 0 else nc.scalar
                    eng.dma_start(
                        vin[:, o], v_inT_t[:, o, blk * PBLK : (blk + 1) * PBLK]
                    )
                for t in range(PBLK // P):
                    lk_t = blk * (PBLK // P) + t
                    for dh in range(D // PBLK):
                        ps = psum_mm.tile([P, PBLK], F32, tag="mm", name="ps_v")
                        for k in range(KCH):
                            nc.tensor.matmul(
                                ps[:],
                                vin[:, k, t * P : (t + 1) * P],
                                Wv_sb[:, k, dh * PBLK : (dh + 1) * PBLK],
                                start=(k == 0),
                                stop=(k == KCH - 1),
                            )
                        nc.vector.tensor_add(
                            v_sb[:, lk_t, dh * PBLK : (dh + 1) * PBLK],
                            ps[:],
                            bv_rep[:, dh * PBLK : (dh + 1) * PBLK],
                        )

        # ---- phase 2: attention with fused q projection ---------------
        if not do_attn:
            return
        Wq_sb = wpool.tile([P, KCH, D], DT, tag="W", name="Wq_sb")
        for o in range(KCH):
            nc.sync.dma_start(Wq_sb[:, o], Wq_t[:, o])
        with tc.tile_pool(name="qin_pool", bufs=2) as qin_pool, tc.tile_pool(
            name="qtb_pool", bufs=2
        ) as qtb_pool, tc.tile_pool(name="osb_pool", bufs=2) as osb_pool, tc.tile_pool(
            name="rsb_pool", bufs=2
        ) as rsb_pool:
            for blk in range(LQS // ABLK):  # 4 blocks of 256 query rows
                qin = qin_pool.tile([P, KCH, ABLK], DT, tag="qin", name="qin")
                for o in range(KCH):
                    eng = nc.sync if o % 2 == 0 else nc.scalar
                    eng.dma_start(
                        qin[:, o], q_inT_t[:, o, blk * ABLK : (blk + 1) * ABLK]
                    )
                # q projection for this block: qtb[do, lq]
                qtb = qtb_pool.tile([P, KCH, ABLK], DT, tag="qtb", name="qtb")
                for m in range(DOT):
                    ps = psum_st.tile([P, ABLK], F32, tag="st", name="ps_qp")
                    for k in range(KCH):
                        nc.tensor.matmul(
                            ps[:],
                            Wq_sb[:, k, m * P : (m + 1) * P],
                            qin[:, k],
                            start=(k == 0),
                            stop=(k == KCH - 1),
                        )
                    nc.vector.tensor_scalar_add(
                        qtb[:, m], ps[:], bq_sb[:, m : m + 1]
                    )
                o_ps = [
                    [
                        psum_mm.tile([P, PBLK], F32, tag="mm", name=f"o_ps_{t}_{dh}")
                        for dh in range(D // PBLK)
                    ]
                    for t in range(ABLK // P)
                ]
                r_ps = [
                    psum_r.tile([P, 2], F32, tag="r", name=f"r_ps_{t}")
                    for t in range(ABLK // P)
                ]
                for c in range(LKT):
                    st = psum_st.tile([P, ABLK], F32, tag="st", name="st")
                    for o in range(KCH):
                        nc.tensor.matmul(
                            st[:],
                            kT_sb[:, o, c * P : (c + 1) * P],
                            qtb[:, o],
                            start=(o == 0),
                            stop=(o == KCH - 1),
                        )
                    pt = pt_pool_tile(tc, name="pt")
                    nc.scalar.activation(
                        pt[:], st[:], mybir.ActivationFunctionType.Exp, scale=SCALE
                    )
                    for t in range(ABLK // P):
                        pt_t = pt[:, t * P : (t + 1) * P]
                        for dh in range(D // PBLK):
                            nc.tensor.matmul(
                                o_ps[t][dh][:],
                                pt_t,
                                v_sb[:, c, dh * PBLK : (dh + 1) * PBLK],
                                start=(c == 0),
                                stop=(c == LKT - 1),
                            )
                        nc.tensor.matmul(
                            r_ps[t][:],
                            pt_t,
                            ones_sb[:],
                            start=(c == 0),
                            stop=(c == LKT - 1),
                        )
                for t in range(ABLK // P):
                    rsb = rsb_pool.tile([P, 1], F32, tag="rsb", name="rsb")
                    nc.vector.reciprocal(rsb[:], r_ps[t][:, 0:1])
                    lq0 = blk * ABLK + t * P
                    for dh in range(D // PBLK):
                        osb = osb_pool.tile([P, PBLK], F32, tag="osb", name="osb")
                        nc.scalar.mul(osb[:], o_ps[t][dh][:], rsb[:])
                        nc.sync.dma_start(
                            out[lq0 : lq0 + P, dh * PBLK : (dh + 1) * PBLK], osb[:]
                        )


_program = None


def _get_program():
    global _program
    if _program is None:
        _program = build_program()
    return _program


def _make_in_maps(query_input, key_input, value_input, Wq, bq, Wk, bk, Wv, bv):
    f32 = np.float32
    Wq_h = _round_host(np.asarray(Wq, HOST_DT))
    Wk_h = _round_host(np.asarray(Wk, HOST_DT))
    Wv_h = _round_host(np.asarray(Wv, HOST_DT))
    bq_h = np.asarray(bq, f32)
    bk_h = np.asarray(bk, f32)
    bv_h = np.asarray(bv, f32)
    in_maps = []
    kT_cache = {}
    for c in range(NCORES):
        b, h = divmod(c, 2)
        if b not in kT_cache:
            kT_cache[b] = (
                _round_host(np.asarray(key_input[b], HOST_DT).T),
                _round_host(np.asarray(value_input[b], HOST_DT).T),
            )
        k_t, v_t = kT_cache[b]
        q_sh = np.asarray(query_input[b, h * LQS : (h + 1) * LQS, :], HOST_DT)
        in_maps.append(
            {
                "q_inT": _round_host(q_sh.T),
                "k_inT": k_t,
                "v_inT": v_t,
                "Wq": Wq_h,
                "Wk": Wk_h,
                "Wv": Wv_h,
                "bq": bq_h,
                "bk": bk_h,
                "bv": bv_h,
            }
        )
    return in_maps


def run(in_maps, **kwargs):
    nc = _get_program()
    return run_bass_kernel_spmd(nc, in_maps, core_ids=list(range(NCORES)), **kwargs)


def kernel(query_input, key_input, value_input, Wq, bq, Wk, bk, Wv, bv):
    in_maps = _make_in_maps(
        query_input, key_input, value_input, Wq, bq, Wk, bk, Wv, bv
    )
    res = run(in_maps)
    out = np.empty((B, LQ, D), np.float32)
    for c in range(NCORES):
        b, h = divmod(c, 2)
        out[b, h * LQS : (h + 1) * LQS, :] = res.results[c]["out"]
    return out



# revision 2
# speedup vs baseline: 730.6736x; 730.6736x over previous
"""Cross-attention kernel for Trainium2, 8 NeuronCores.

Problem (full shapes): B=4, Lq=Lk=2048, D(in)=D(out)=1024, fp32.
    q = query @ Wq + bq ; k = key @ Wk + bk ; v = value @ Wv + bv
    out = softmax(q k^T / sqrt(1024)) v

Sharding: 8 cores = (batch b, half h of Lq). Each core computes a
[1024, 1024] slice of the output for batch b, query rows
[h*1024, (h+1)*1024). K/V projections for a batch are duplicated across
the core pair (cheaper than cross-core exchange for this size).

v2 design notes (vs the f32r baseline):
  - All matmul operands are bf16 (fp32 PSUM accumulation).  Same PE
    throughput as f32r (1 cyc/row) but half the DMA bytes and half the
    SBUF footprint, which buys room to keep all three weight matrices
    resident and prefetch every phase's inputs ahead of the compute.
  - DMA placement: the two HWDGE rings are used as streams — SP ring
    carries Wk, kin[0..3], Wv, vin[0..3] (phase 0/1); ACT ring carries
    Wq, qin prefetches and the output blocks.  Within a ring,
    descriptors are emitted in consumption order so head-of-line waits
    are always released before the data is needed; across repeat
    bodies the next body's phase-0 loads stream during this body's
    attention (the repeat seam is fully hidden).
  - Host supplies inputs pre-chunked ([blk][p][o][l]) so every DMA is
    one descriptor with 4-16 KB contiguous runs per partition.
  - The attention c-loop is software-pipelined by one iteration
    (scores(c+1) issues on PE before pv(c)), so the Exp on the ACT
    engine never stalls the PE.
  - Scores are small (|S/32| < ~3) so exp is applied without
    max-subtraction; row sums come from a PE matmul with a ones
    column; normalization is a per-partition scalar multiply.  v
    carries its bias, so softmax rows summing to 1 makes the output
    bias exact.
"""

import os
import sys

sys.path.insert(0, "/opt/trn_rl_repo")

from contextlib import ExitStack

import numpy as np

import concourse.bass as bass
import concourse.tile as tile
from concourse import bacc, mybir
from concourse.bass_utils import run_bass_kernel_spmd

P = 128
B, LQ, LK, D = 4, 2048, 2048, 1024
NCORES = 8
LQS = LQ * B // NCORES  # 1024 query rows per core
KCH = D // P  # 8 contraction chunks
DOT = D // P  # 8 output-feature tiles
LKT = LK // P  # 16 key tiles
PBLK = 512  # projection matmul free dim
ABLK = 256  # attention lq block (2 lq tiles)
NBK = LK // PBLK  # 4 key/value input blocks
NBQ = LQS // ABLK  # 4 query input blocks
SCALE = 1.0 / 32.0  # 1/sqrt(D)

F32 = mybir.dt.float32
DT = mybir.dt.bfloat16

import ml_dtypes

HOST_DT = np.dtype(ml_dtypes.bfloat16)


def build_program(repeat=1):
    nc = bacc.Bacc("TRN2", target_bir_lowering=False, debug=False)

    q_in_c = nc.dram_tensor("q_in_c", [NBQ, P, KCH, ABLK], DT, kind="ExternalInput").ap()
    k_in_c = nc.dram_tensor("k_in_c", [NBK, P, KCH, PBLK], DT, kind="ExternalInput").ap()
    v_in_c = nc.dram_tensor("v_in_c", [NBK, P, KCH, PBLK], DT, kind="ExternalInput").ap()
    Wq_c = nc.dram_tensor("Wq_c", [P, KCH, D], DT, kind="ExternalInput").ap()
    Wk_c = nc.dram_tensor("Wk_c", [P, KCH, D], DT, kind="ExternalInput").ap()
    Wv_c = nc.dram_tensor("Wv_c", [P, KCH, D], DT, kind="ExternalInput").ap()
    bq = nc.dram_tensor("bq", [D], F32, kind="ExternalInput").ap()
    bk = nc.dram_tensor("bk", [D], F32, kind="ExternalInput").ap()
    bv = nc.dram_tensor("bv", [D], F32, kind="ExternalInput").ap()
    out = nc.dram_tensor("out", [LQS, D], F32, kind="ExternalOutput").ap()
    out_r = out.rearrange("(blk t p) d -> blk p t d", t=ABLK // P, p=P)

    with tile.TileContext(nc) as tc, ExitStack() as ctx:
        pool = lambda name, bufs, **kw: ctx.enter_context(
            tc.tile_pool(name=name, bufs=bufs, **kw)
        )
        psum_mm = pool("psum_mm", 4, space="PSUM")
        psum_st = pool("psum_st", 2, space="PSUM")
        psum_r = pool("psum_r", 2, space="PSUM")
        singles = pool("singles", 1)
        kt_pool = pool("kt_pool", 1)
        v_pool = pool("v_pool", 1)
        w_pool = pool("w_pool", 3)
        kin_pool = pool("kin_pool", 3)
        vin_pool = pool("vin_pool", 3)
        qin_pool = pool("qin_pool", 2)
        qtb_pool = pool("qtb_pool", 2)
        pt_pool = pool("pt_pool", 6)
        osb_pool = pool("osb_pool", 2)
        rsb_pool = pool("rsb_pool", 2)

        # ---- constants -------------------------------------------------
        bq_sb = singles.tile([P, DOT], F32, name="bq_sb")
        nc.sync.dma_start(bq_sb[:], bq.rearrange("(o p) -> p o", p=P))
        bk_sb = singles.tile([P, DOT], F32, name="bk_sb")
        nc.sync.dma_start(bk_sb[:], bk.rearrange("(o p) -> p o", p=P))
        # bv broadcast to all 128 partitions (stride-0 partition read)
        bv_rep = singles.tile([P, D], F32, name="bv_rep")
        bv_bcast = bass.AP(tensor=bv.tensor, offset=bv.offset, ap=[[0, P], *bv.ap])
        nc.gpsimd.dma_start(bv_rep[:], bv_bcast)
        ones_sb = singles.tile([P, 2], DT, name="ones_sb")
        nc.vector.memset(ones_sb[:], 1.0)

        pools = (psum_mm, psum_st, psum_r, kt_pool, v_pool, w_pool,
                 kin_pool, vin_pool, qin_pool, qtb_pool, pt_pool,
                 osb_pool, rsb_pool)
        consts = (bq_sb, bk_sb, bv_rep, ones_sb)
        tensors = (q_in_c, k_in_c, v_in_c, Wq_c, Wk_c, Wv_c, out_r)
        for _rep in range(repeat):
            one_pass(nc, pools, consts, tensors)

    nc.compile()
    return nc


def one_pass(nc, pools, consts, tensors):
    (psum_mm, psum_st, psum_r, kt_pool, v_pool, w_pool,
     kin_pool, vin_pool, qin_pool, qtb_pool, pt_pool,
     osb_pool, rsb_pool) = pools
    bq_sb, bk_sb, bv_rep, ones_sb = consts
    q_in_c, k_in_c, v_in_c, Wq_c, Wk_c, Wv_c, out_r = tensors

    kT_sb = kt_pool.tile([P, DOT, LK], DT, tag="kT", name="kT_sb")
    v_sb = v_pool.tile([P, LKT, D], DT, tag="v", name="v_sb")

    # ---- DMA issue: SP ring streams phase 0+1, ACT ring phase 2 -------
    Wk_sb = w_pool.tile([P, KCH, D], DT, tag="W", name="Wk_sb")
    nc.sync.dma_start(Wk_sb[:], Wk_c[:])
    kin = []
    for n in range(NBK):
        t = kin_pool.tile([P, KCH, PBLK], DT, tag="kin", name=f"kin{n}")
        nc.sync.dma_start(t[:], k_in_c[n])
        kin.append(t)
    Wv_sb = w_pool.tile([P, KCH, D], DT, tag="W", name="Wv_sb")
    nc.sync.dma_start(Wv_sb[:], Wv_c[:])
    vin = []
    for n in range(NBK):
        t = vin_pool.tile([P, KCH, PBLK], DT, tag="vin", name=f"vin{n}")
        nc.sync.dma_start(t[:], v_in_c[n])
        vin.append(t)
    Wq_sb = w_pool.tile([P, KCH, D], DT, tag="W", name="Wq_sb")
    nc.scalar.dma_start(Wq_sb[:], Wq_c[:])
    qin = [None] * NBQ
    for n in range(2):  # first two q blocks prefetched up front
        qin[n] = qin_pool.tile([P, KCH, ABLK], DT, tag="qin", name=f"qin{n}")
        nc.scalar.dma_start(qin[n][:], q_in_c[n])

    # ---- phase 0: kT = (k_in @ Wk + bk)^T, SBUF-resident --------------
    for n in range(NBK):
        for m in range(DOT):
            ps = psum_mm.tile([P, PBLK], F32, tag="mm", name="ps_k")
            for k in range(KCH):
                nc.tensor.matmul(
                    ps[:],
                    Wk_sb[:, k, m * P : (m + 1) * P],
                    kin[n][:, k],
                    start=(k == 0),
                    stop=(k == KCH - 1),
                )
            nc.vector.tensor_scalar_add(
                kT_sb[:, m, n * PBLK : (n + 1) * PBLK],
                ps[:],
                bk_sb[:, m : m + 1],
            )

    # ---- phase 1: v = v_in @ Wv + bv, natural layout, resident --------
    for n in range(NBK):
        for t in range(PBLK // P):
            lk_t = n * (PBLK // P) + t
            for dh in range(D // PBLK):
                ps = psum_mm.tile([P, PBLK], F32, tag="mm", name="ps_v")
                for k in range(KCH):
                    nc.tensor.matmul(
                        ps[:],
                        vin[n][:, k, t * P : (t + 1) * P],
                        Wv_sb[:, k, dh * PBLK : (dh + 1) * PBLK],
                        start=(k == 0),
                        stop=(k == KCH - 1),
                    )
                nc.vector.tensor_add(
                    v_sb[:, lk_t, dh * PBLK : (dh + 1) * PBLK],
                    ps[:],
                    bv_rep[:, dh * PBLK : (dh + 1) * PBLK],
                )

    # ---- phase 2: attention with fused q projection -------------------
    for blk in range(NBQ):
        if blk + 2 < NBQ:
            qin[blk + 2] = qin_pool.tile(
                [P, KCH, ABLK], DT, tag="qin", name=f"qin{blk + 2}"
            )
            nc.scalar.dma_start(qin[blk + 2][:], q_in_c[blk + 2])
        # q projection for this block: qtb[do, lq]
        qtb = qtb_pool.tile([P, KCH, ABLK], DT, tag="qtb", name="qtb")
        for m in range(DOT):
            ps = psum_st.tile([P, ABLK], F32, tag="st", name="ps_qp")
            for k in range(KCH):
                nc.tensor.matmul(
                    ps[:],
                    Wq_sb[:, k, m * P : (m + 1) * P],
                    qin[blk][:, k],
                    start=(k == 0),
                    stop=(k == KCH - 1),
                )
            nc.vector.tensor_scalar_add(qtb[:, m], ps[:], bq_sb[:, m : m + 1])

        o_ps = [
            [
                psum_mm.tile([P, PBLK], F32, tag="mm", name=f"o_ps_{t}_{dh}")
                for dh in range(D // PBLK)
            ]
            for t in range(ABLK // P)
        ]
        r_ps = [
            psum_r.tile([P, 2], F32, tag="r", name=f"r_ps_{t}")
            for t in range(ABLK // P)
        ]

        def scores(c):
            st = psum_st.tile([P, ABLK], F32, tag="st", name="st")
            for o in range(KCH):
                nc.tensor.matmul(
                    st[:],
                    kT_sb[:, o, c * P : (c + 1) * P],
                    qtb[:, o],
                    start=(o == 0),
                    stop=(o == KCH - 1),
                )
            return st

        # c-loop software-pipelined by one: scores(c+1) issues on PE
        # before pv(c), so exp(c) on ACT overlaps scores(c+1).
        st_cur = scores(0)
        for c in range(LKT):
            pt = pt_pool.tile([P, ABLK], DT, tag="pt", name="pt")
            nc.scalar.activation(
                pt[:], st_cur[:], mybir.ActivationFunctionType.Exp, scale=SCALE
            )
            if c + 1 < LKT:
                st_cur = scores(c + 1)
            for t in range(ABLK // P):
                pt_t = pt[:, t * P : (t + 1) * P]
                for dh in range(D // PBLK):
                    nc.tensor.matmul(
                        o_ps[t][dh][:],
                        pt_t,
                        v_sb[:, c, dh * PBLK : (dh + 1) * PBLK],
                        start=(c == 0),
                        stop=(c == LKT - 1),
                    )
                nc.tensor.matmul(
                    r_ps[t][:],
                    pt_t,
                    ones_sb[:],
                    start=(c == 0),
                    stop=(c == LKT - 1),
                )

        osb = osb_pool.tile([P, ABLK // P, D], F32, tag="osb", name="osb")
        for t in range(ABLK // P):
            rsb = rsb_pool.tile([P, 1], F32, tag="rsb", name="rsb")
            nc.vector.reciprocal(rsb[:], r_ps[t][:, 0:1])
            for dh in range(D // PBLK):
                nc.scalar.mul(
                    osb[:, t, dh * PBLK : (dh + 1) * PBLK], o_ps[t][dh][:], rsb[:]
                )
        nc.scalar.dma_start(out_r[blk], osb[:])


_program = None


def _get_program():
    global _program
    if _program is None:
        _program = build_program()
    return _program


def _chunk_lin(x, blk):
    """[L, Din] -> [L//blk, P, KCH, blk] with [n,p,o,l] = x[n*blk+l, o*P+p]."""
    L = x.shape[0]
    return np.ascontiguousarray(
        x.reshape(L // blk, blk, KCH, P).transpose(0, 3, 2, 1)
    )


def _chunk_w(w):
    """[Din, Dout] -> [P, KCH, Dout] with [p,o,n] = w[o*P+p, n]."""
    return np.ascontiguousarray(w.reshape(KCH, P, D).transpose(1, 0, 2))


def _make_in_maps(query_input, key_input, value_input, Wq, bq, Wk, bk, Wv, bv):
    f32 = np.float32
    Wq_h = _chunk_w(np.asarray(Wq, HOST_DT))
    Wk_h = _chunk_w(np.asarray(Wk, HOST_DT))
    Wv_h = _chunk_w(np.asarray(Wv, HOST_DT))
    bq_h = np.asarray(bq, f32)
    bk_h = np.asarray(bk, f32)
    bv_h = np.asarray(bv, f32)
    in_maps = []
    kv_cache = {}
    for c in range(NCORES):
        b, h = divmod(c, 2)
        if b not in kv_cache:
            kv_cache[b] = (
                _chunk_lin(np.asarray(key_input[b], HOST_DT), PBLK),
                _chunk_lin(np.asarray(value_input[b], HOST_DT), PBLK),
            )
        k_c, v_c = kv_cache[b]
        q_c = _chunk_lin(
            np.asarray(query_input[b, h * LQS : (h + 1) * LQS, :], HOST_DT), ABLK
        )
        in_maps.append(
            {
                "q_in_c": q_c,
                "k_in_c": k_c,
                "v_in_c": v_c,
                "Wq_c": Wq_h,
                "Wk_c": Wk_h,
                "Wv_c": Wv_h,
                "bq": bq_h,
                "bk": bk_h,
                "bv": bv_h,
            }
        )
    return in_maps


def run(in_maps, **kwargs):
    nc = _get_program()
    return run_bass_kernel_spmd(nc, in_maps, core_ids=list(range(NCORES)), **kwargs)


def kernel(query_input, key_input, value_input, Wq, bq, Wk, bk, Wv, bv):
    in_maps = _make_in_maps(
        query_input, key_input, value_input, Wq, bq, Wk, bk, Wv, bv
    )
    res = run(in_maps)
    out = np.empty((B, LQ, D), np.float32)
    for c in range(NCORES):
        b, h = divmod(c, 2)
        out[b, h * LQS : (h + 1) * LQS, :] = res.results[c]["out"]
    return out


# revision 7
# speedup vs baseline: 747.7019x; 1.0233x over previous
"""Cross-attention kernel for Trainium2, 8 NeuronCores.

Problem (full shapes): B=4, Lq=Lk=2048, D(in)=D(out)=1024, fp32.
    q = query @ Wq + bq ; k = key @ Wk + bk ; v = value @ Wv + bv
    out = softmax(q k^T / sqrt(1024)) v

Sharding: 8 cores = (batch b, half h of Lq). Each core computes a
[1024, 1024] slice of the output for batch b, query rows
[h*1024, (h+1)*1024). K/V projections for a batch are duplicated across
the core pair (cheaper than cross-core exchange for this size).

v2 design notes (vs the f32r baseline):
  - All matmul operands are bf16 (fp32 PSUM accumulation).  Same PE
    throughput as f32r (1 cyc/row) but half the DMA bytes and half the
    SBUF footprint, which buys room to keep all three weight matrices
    resident and prefetch every phase's inputs ahead of the compute.
  - DMA placement: the two HWDGE rings are used as streams — SP ring
    carries Wk, kin[0..3], Wv, vin[0..3] (phase 0/1); ACT ring carries
    Wq, qin prefetches and the output blocks.  Within a ring,
    descriptors are emitted in consumption order so head-of-line waits
    are always released before the data is needed; across repeat
    bodies the next body's phase-0 loads stream during this body's
    attention (the repeat seam is fully hidden).
  - Host supplies inputs pre-chunked ([blk][p][o][l]) so every DMA is
    one descriptor with 4-16 KB contiguous runs per partition.
  - The attention c-loop is software-pipelined by one iteration
    (scores(c+1) issues on PE before pv(c)), so the Exp on the ACT
    engine never stalls the PE.
  - Scores are small (|S/32| < ~3) so exp is applied without
    max-subtraction; row sums come from a PE matmul with a ones
    column; normalization is a per-partition scalar multiply.  v
    carries its bias, so softmax rows summing to 1 makes the output
    bias exact.
"""

import os
import sys

sys.path.insert(0, "/opt/trn_rl_repo")

from contextlib import ExitStack

import numpy as np

import concourse.bass as bass
import concourse.tile as tile
from concourse import bacc, mybir
from concourse.bass_utils import run_bass_kernel_spmd

P = 128
B, LQ, LK, D = 4, 2048, 2048, 1024
NCORES = 8
LQS = LQ * B // NCORES  # 1024 query rows per core
KCH = D // P  # 8 contraction chunks
DOT = D // P  # 8 output-feature tiles
LKT = LK // P  # 16 key tiles
PBLK = 512  # projection matmul free dim
ABLK = 256  # attention lq block (2 lq tiles)
NBK = LK // PBLK  # 4 key/value input blocks
NBQ = LQS // ABLK  # 4 query input blocks
SCALE = 1.0 / 32.0  # 1/sqrt(D)

F32 = mybir.dt.float32
DT = mybir.dt.bfloat16

import ml_dtypes

HOST_DT = np.dtype(ml_dtypes.bfloat16)


def build_program(repeat=1):
    nc = bacc.Bacc("TRN2", target_bir_lowering=False, debug=False)

    q_in_c = nc.dram_tensor("q_in_c", [NBQ, P, KCH, ABLK], DT, kind="ExternalInput").ap()
    k_in_c = nc.dram_tensor("k_in_c", [NBK, P, KCH, PBLK], DT, kind="ExternalInput").ap()
    v_in_c = nc.dram_tensor("v_in_c", [NBK, P, KCH, PBLK], DT, kind="ExternalInput").ap()
    Wq_c = nc.dram_tensor("Wq_c", [P, KCH, D], DT, kind="ExternalInput").ap()
    Wk_c = nc.dram_tensor("Wk_c", [P, KCH, D], DT, kind="ExternalInput").ap()
    Wv_c = nc.dram_tensor("Wv_c", [P, KCH, D], DT, kind="ExternalInput").ap()
    bq = nc.dram_tensor("bq", [D], F32, kind="ExternalInput").ap()
    bk = nc.dram_tensor("bk", [D], F32, kind="ExternalInput").ap()
    bv = nc.dram_tensor("bv", [D], F32, kind="ExternalInput").ap()
    out = nc.dram_tensor("out", [LQS, D], F32, kind="ExternalOutput").ap()
    out_r = out.rearrange("(blk t p) d -> blk p t d", t=ABLK // P, p=P)

    with tile.TileContext(nc) as tc, ExitStack() as ctx:
        pool = lambda name, bufs, **kw: ctx.enter_context(
            tc.tile_pool(name=name, bufs=bufs, **kw)
        )
        psum_mm = pool("psum_mm", 4, space="PSUM")
        psum_st = pool("psum_st", 2, space="PSUM")
        psum_r = pool("psum_r", 2, space="PSUM")
        singles = pool("singles", 1)
        kt_pool = pool("kt_pool", 1)
        v_pool = pool("v_pool", 1)
        w_pool = pool("w_pool", 3)
        kin_pool = pool("kin_pool", 3)
        vin_pool = pool("vin_pool", 3)
        qin_pool = pool("qin_pool", 2)
        qtb_pool = pool("qtb_pool", 2)
        pt_pool = pool("pt_pool", 6)
        osb_pool = pool("osb_pool", 2)
        rsb_pool = pool("rsb_pool", 2)

        # ---- constants -------------------------------------------------
        bq_sb = singles.tile([P, DOT], F32, name="bq_sb")
        nc.sync.dma_start(bq_sb[:], bq.rearrange("(o p) -> p o", p=P))
        bk_sb = singles.tile([P, DOT], F32, name="bk_sb")
        nc.sync.dma_start(bk_sb[:], bk.rearrange("(o p) -> p o", p=P))
        # bv broadcast to all 128 partitions (stride-0 partition read)
        bv_rep = singles.tile([P, D], F32, name="bv_rep")
        bv_bcast = bass.AP(tensor=bv.tensor, offset=bv.offset, ap=[[0, P], *bv.ap])
        nc.gpsimd.dma_start(bv_rep[:], bv_bcast)
        ones_sb = singles.tile([P, 2], DT, name="ones_sb")
        nc.vector.memset(ones_sb[:], 1.0)

        pools = (psum_mm, psum_st, psum_r, kt_pool, v_pool, w_pool,
                 kin_pool, vin_pool, qin_pool, qtb_pool, pt_pool,
                 osb_pool, rsb_pool)
        consts = (bq_sb, bk_sb, bv_rep, ones_sb)
        tensors = (q_in_c, k_in_c, v_in_c, Wq_c, Wk_c, Wv_c, out_r)
        phases = os.environ.get("KPHASES", "all")
        for _rep in range(repeat):
            one_pass(nc, pools, consts, tensors, phases=phases)

    nc.compile()
    return nc


def one_pass(nc, pools, consts, tensors, phases="all"):
    (psum_mm, psum_st, psum_r, kt_pool, v_pool, w_pool,
     kin_pool, vin_pool, qin_pool, qtb_pool, pt_pool,
     osb_pool, rsb_pool) = pools
    bq_sb, bk_sb, bv_rep, ones_sb = consts
    q_in_c, k_in_c, v_in_c, Wq_c, Wk_c, Wv_c, out_r = tensors

    do_proj = phases in ("all", "proj")
    do_attn = phases in ("all", "attn")
    kT_sb = kt_pool.tile([P, DOT, LK], DT, tag="kT", name="kT_sb")
    v_sb = v_pool.tile([P, LKT, D], DT, tag="v", name="v_sb")
    if not do_proj:
        nc.vector.memset(kT_sb[:, 0, 0:8], 0.0)
        nc.vector.memset(v_sb[:, 0, 0:8], 0.0)

    # ---- DMA issue: SP ring streams phase 0+1, ACT ring phase 2 -------
    Wk_sb = w_pool.tile([P, KCH, D], DT, tag="W", name="Wk_sb")
    nc.sync.dma_start(Wk_sb[:], Wk_c[:])
    kin = []
    for n in range(NBK):
        t = kin_pool.tile([P, KCH, PBLK], DT, tag="kin", name=f"kin{n}")
        nc.sync.dma_start(t[:], k_in_c[n])
        kin.append(t)
    Wv_sb = w_pool.tile([P, KCH, D], DT, tag="W", name="Wv_sb")
    nc.sync.dma_start(Wv_sb[:], Wv_c[:])
    vin = []
    for n in range(NBK):
        t = vin_pool.tile([P, KCH, PBLK], DT, tag="vin", name=f"vin{n}")
        nc.sync.dma_start(t[:], v_in_c[n])
        vin.append(t)
    Wq_sb = w_pool.tile([P, KCH, D], DT, tag="W", name="Wq_sb")
    nc.scalar.dma_start(Wq_sb[:], Wq_c[:])
    qin = [None] * NBQ
    for n in range(2):  # first two q blocks prefetched up front
        qin[n] = qin_pool.tile([P, KCH, ABLK], DT, tag="qin", name=f"qin{n}")
        nc.scalar.dma_start(qin[n][:], q_in_c[n])

    # ---- phase 0: kT = (k_in @ Wk + bk)^T, SBUF-resident --------------
    for n in range(NBK if do_proj else 0):
        for m in range(DOT):
            ps = psum_mm.tile([P, PBLK], F32, tag="mm", name="ps_k")
            for k in range(KCH):
                nc.tensor.matmul(
                    ps[:],
                    Wk_sb[:, k, m * P : (m + 1) * P],
                    kin[n][:, k],
                    start=(k == 0),
                    stop=(k == KCH - 1),
                )
            nc.vector.tensor_scalar_add(
                kT_sb[:, m, n * PBLK : (n + 1) * PBLK],
                ps[:],
                bk_sb[:, m : m + 1],
            )

    # ---- phase 1: v = v_in @ Wv + bv, natural layout, resident --------
    for n in range(NBK if do_proj else 0):
        for t in range(PBLK // P):
            lk_t = n * (PBLK // P) + t
            for dh in range(D // PBLK):
                ps = psum_mm.tile([P, PBLK], F32, tag="mm", name="ps_v")
                for k in range(KCH):
                    nc.tensor.matmul(
                        ps[:],
                        vin[n][:, k, t * P : (t + 1) * P],
                        Wv_sb[:, k, dh * PBLK : (dh + 1) * PBLK],
                        start=(k == 0),
                        stop=(k == KCH - 1),
                    )
                nc.vector.tensor_add(
                    v_sb[:, lk_t, dh * PBLK : (dh + 1) * PBLK],
                    ps[:],
                    bv_rep[:, dh * PBLK : (dh + 1) * PBLK],
                )

    # ---- phase 2: attention with fused q projection -------------------
    for blk in range(NBQ if do_attn else 0):
        if blk + 2 < NBQ:
            qin[blk + 2] = qin_pool.tile(
                [P, KCH, ABLK], DT, tag="qin", name=f"qin{blk + 2}"
            )
            nc.scalar.dma_start(qin[blk + 2][:], q_in_c[blk + 2])
        # q projection for this block: qtb[do, lq]
        qtb = qtb_pool.tile([P, KCH, ABLK], DT, tag="qtb", name="qtb")
        for m in range(DOT):
            ps = psum_st.tile([P, ABLK], F32, tag="st", name="ps_qp")
            for k in range(KCH):
                nc.tensor.matmul(
                    ps[:],
                    Wq_sb[:, k, m * P : (m + 1) * P],
                    qin[blk][:, k],
                    start=(k == 0),
                    stop=(k == KCH - 1),
                )
            nc.vector.tensor_scalar_add(qtb[:, m], ps[:], bq_sb[:, m : m + 1])

        o_ps = [
            [
                psum_mm.tile([P, PBLK], F32, tag="mm", name=f"o_ps_{t}_{dh}")
                for dh in range(D // PBLK)
            ]
            for t in range(ABLK // P)
        ]
        r_ps = [
            psum_r.tile([P, 2], F32, tag="r", name=f"r_ps_{t}")
            for t in range(ABLK // P)
        ]

        # All scores (+exp into SBUF) first, then all pv: the exp of
        # chunk c runs on ACT while the PE is still ~15 score-groups
        # away from needing pt[c], so cross-engine latency never stalls
        # the PE (per-iteration scores->exp->pv chaining did).
        pts = []
        for c in range(LKT):
            st = psum_st.tile([P, ABLK], F32, tag="st", name="st")
            for o in range(KCH):
                nc.tensor.matmul(
                    st[:],
                    kT_sb[:, o, c * P : (c + 1) * P],
                    qtb[:, o],
                    start=(o == 0),
                    stop=(o == KCH - 1),
                )
            pt = pt_pool.tile([P, ABLK], DT, tag="pt", name="pt")
            nc.scalar.activation(
                pt[:], st[:], mybir.ActivationFunctionType.Exp, scale=SCALE
            )
            pts.append(pt)
        for c in range(LKT):
            pt = pts[c]
            for t in range(ABLK // P):
                pt_t = pt[:, t * P : (t + 1) * P]
                for dh in range(D // PBLK):
                    nc.tensor.matmul(
                        o_ps[t][dh][:],
                        pt_t,
                        v_sb[:, c, dh * PBLK : (dh + 1) * PBLK],
                        start=(c == 0),
                        stop=(c == LKT - 1),
                    )
                nc.tensor.matmul(
                    r_ps[t][:],
                    pt_t,
                    ones_sb[:],
                    start=(c == 0),
                    stop=(c == LKT - 1),
                )

        osb = osb_pool.tile([P, ABLK // P, D], F32, tag="osb", name="osb")
        for t in range(ABLK // P):
            rsb = rsb_pool.tile([P, 1], F32, tag="rsb", name="rsb")
            nc.vector.reciprocal(rsb[:], r_ps[t][:, 0:1])
            for dh in range(D // PBLK):
                nc.vector.tensor_scalar_mul(
                    osb[:, t, dh * PBLK : (dh + 1) * PBLK], o_ps[t][dh][:], rsb[:]
                )
        nc.scalar.dma_start(out_r[blk], osb[:])


_program = None


def _get_program():
    global _program
    if _program is None:
        _program = build_program()
    return _program


def _chunk_lin(x, blk):
    """[L, Din] -> [L//blk, P, KCH, blk] with [n,p,o,l] = x[n*blk+l, o*P+p]."""
    L = x.shape[0]
    return np.ascontiguousarray(
        x.reshape(L // blk, blk, KCH, P).transpose(0, 3, 2, 1)
    )


def _chunk_w(w):
    """[Din, Dout] -> [P, KCH, Dout] with [p,o,n] = w[o*P+p, n]."""
    return np.ascontiguousarray(w.reshape(KCH, P, D).transpose(1, 0, 2))


def _make_in_maps(query_input, key_input, value_input, Wq, bq, Wk, bk, Wv, bv):
    f32 = np.float32
    Wq_h = _chunk_w(np.asarray(Wq, HOST_DT))
    Wk_h = _chunk_w(np.asarray(Wk, HOST_DT))
    Wv_h = _chunk_w(np.asarray(Wv, HOST_DT))
    bq_h = np.asarray(bq, f32)
    bk_h = np.asarray(bk, f32)
    bv_h = np.asarray(bv, f32)
    in_maps = []
    kv_cache = {}
    for c in range(NCORES):
        b, h = divmod(c, 2)
        if b not in kv_cache:
            kv_cache[b] = (
                _chunk_lin(np.asarray(key_input[b], HOST_DT), PBLK),
                _chunk_lin(np.asarray(value_input[b], HOST_DT), PBLK),
            )
        k_c, v_c = kv_cache[b]
        q_c = _chunk_lin(
            np.asarray(query_input[b, h * LQS : (h + 1) * LQS, :], HOST_DT), ABLK
        )
        in_maps.append(
            {
                "q_in_c": q_c,
                "k_in_c": k_c,
                "v_in_c": v_c,
                "Wq_c": Wq_h,
                "Wk_c": Wk_h,
                "Wv_c": Wv_h,
                "bq": bq_h,
                "bk": bk_h,
                "bv": bv_h,
            }
        )
    return in_maps


def run(in_maps, **kwargs):
    nc = _get_program()
    return run_bass_kernel_spmd(nc, in_maps, core_ids=list(range(NCORES)), **kwargs)


def kernel(query_input, key_input, value_input, Wq, bq, Wk, bk, Wv, bv):
    in_maps = _make_in_maps(
        query_input, key_input, value_input, Wq, bq, Wk, bk, Wv, bv
    )
    res = run(in_maps)
    out = np.empty((B, LQ, D), np.float32)
    for c in range(NCORES):
        b, h = divmod(c, 2)
        out[b, h * LQS : (h + 1) * LQS, :] = res.results[c]["out"]
    return out


# revision 10
# speedup vs baseline: 842.6211x; 1.1269x over previous
"""Cross-attention kernel for Trainium2, 8 NeuronCores — v3.

v3 = v2 (bf16 operands, full weight residency, ring-disciplined DMA,
software-pipelined attention) + pair-wise K/V projection dedup:

Each core projects only ITS OWN half of the keys/values (1024 rows),
writes the result to a DRAM bounce buffer, and a pair AllGather
({0,1},{2,3},{4,5},{6,7}) assembles both halves.  Both cores then read
the gathered halves back in rank order, so SBUF key order is identical
on both cores (no core-dependent addressing) — softmax without
max-subtraction is permutation-invariant over keys, so ordering is
free.  This cuts per-core PE work from 19.3 to 15.0 GFLOP
(246 -> 191 us roofline).

Phase order per body: [kT proj own + AG_k] [v proj own + AG_v]
[q proj all 4 blocks] [attention].  The q-projection phase exists to
give AG_v a ~28 us window before the first PV matmul needs v.
"""

import os
import sys

sys.path.insert(0, "/opt/trn_rl_repo")

from contextlib import ExitStack

import numpy as np

import concourse.bass as bass
import concourse.tile as tile
from concourse import bacc, mybir
from concourse.bass_utils import run_bass_kernel_spmd

P = 128
B, LQ, LK, D = 4, 2048, 2048, 1024
NCORES = 8
LQS = LQ * B // NCORES  # 1024 query rows per core
LKS = LK // 2  # 1024 own key rows per core
KCH = D // P  # 8 contraction chunks
DOT = D // P  # 8 output-feature tiles
LKT = LK // P  # 16 key tiles (8 per half)
PBLK = 512  # projection matmul free dim
ABLK = 256  # attention lq block (2 lq tiles)
NBK = LKS // PBLK  # 2 own key/value input blocks
NBQ = LQS // ABLK  # 4 query input blocks
SCALE = 1.0 / 32.0  # 1/sqrt(D)

F32 = mybir.dt.float32
DT = mybir.dt.bfloat16

import ml_dtypes

HOST_DT = np.dtype(ml_dtypes.bfloat16)


def build_program(repeat=1, n_cores=NCORES):
    nc = bacc.Bacc(
        "TRN2", target_bir_lowering=False, debug=False, num_devices=n_cores
    )
    groups = [[i, i + 1] for i in range(0, n_cores, 2)]

    q_in_c = nc.dram_tensor("q_in_c", [NBQ, P, KCH, ABLK], DT, kind="ExternalInput").ap()
    k_in_c = nc.dram_tensor("k_in_c", [NBK, P, KCH, PBLK], DT, kind="ExternalInput").ap()
    v_in_c = nc.dram_tensor("v_in_c", [NBK, P, KCH, PBLK], DT, kind="ExternalInput").ap()
    Wq_c = nc.dram_tensor("Wq_c", [P, KCH, D], DT, kind="ExternalInput").ap()
    Wk_c = nc.dram_tensor("Wk_c", [P, KCH, D], DT, kind="ExternalInput").ap()
    Wv_c = nc.dram_tensor("Wv_c", [P, KCH, D], DT, kind="ExternalInput").ap()
    bq = nc.dram_tensor("bq", [D], F32, kind="ExternalInput").ap()
    bk = nc.dram_tensor("bk", [D], F32, kind="ExternalInput").ap()
    bv = nc.dram_tensor("bv", [D], F32, kind="ExternalInput").ap()
    out = nc.dram_tensor("out", [LQS, D], F32, kind="ExternalOutput").ap()
    out_r = out.rearrange("(blk t p) d -> blk p t d", t=ABLK // P, p=P)

    with tile.TileContext(nc) as tc, ExitStack() as ctx:
        pool = lambda name, bufs, **kw: ctx.enter_context(
            tc.tile_pool(name=name, bufs=bufs, **kw)
        )
        psum_mm = pool("psum_mm", 4, space="PSUM")
        psum_st = pool("psum_st", 2, space="PSUM")
        psum_r = pool("psum_r", 2, space="PSUM")
        dram = pool("dram", 2, space="DRAM")
        singles = pool("singles", 1)
        kt_pool = pool("kt_pool", 1)
        v_pool = pool("v_pool", 1)
        w_pool = pool("w_pool", 3)
        kin_pool = pool("kin_pool", 2)
        vin_pool = pool("vin_pool", 2)
        qin_pool = pool("qin_pool", 2)
        qtb_pool = pool("qtb_pool", 4)
        pt_pool = pool("pt_pool", 16)
        osb_pool = pool("osb_pool", 1)
        rsb_pool = pool("rsb_pool", 2)
        stage_pool = pool("stage_pool", 2)

        # ---- constants -------------------------------------------------
        bq_sb = singles.tile([P, DOT], F32, name="bq_sb")
        nc.sync.dma_start(bq_sb[:], bq.rearrange("(o p) -> p o", p=P))
        bk_sb = singles.tile([P, DOT], F32, name="bk_sb")
        nc.sync.dma_start(bk_sb[:], bk.rearrange("(o p) -> p o", p=P))
        bv_rep = singles.tile([P, D], F32, name="bv_rep")
        bv_bcast = bass.AP(tensor=bv.tensor, offset=bv.offset, ap=[[0, P], *bv.ap])
        nc.gpsimd.dma_start(bv_rep[:], bv_bcast)
        ones_sb = singles.tile([P, 2], DT, name="ones_sb")
        nc.vector.memset(ones_sb[:], 1.0)

        pools = (psum_mm, psum_st, psum_r, dram, kt_pool, v_pool, w_pool,
                 kin_pool, vin_pool, qin_pool, qtb_pool, pt_pool,
                 osb_pool, rsb_pool, stage_pool)
        consts = (bq_sb, bk_sb, bv_rep, ones_sb)
        tensors = (q_in_c, k_in_c, v_in_c, Wq_c, Wk_c, Wv_c, out_r)
        for _rep in range(repeat):
            one_pass(nc, pools, consts, tensors, groups)

    nc.compile()
    return nc


def one_pass(nc, pools, consts, tensors, groups):
    (psum_mm, psum_st, psum_r, dram, kt_pool, v_pool, w_pool,
     kin_pool, vin_pool, qin_pool, qtb_pool, pt_pool,
     osb_pool, rsb_pool, stage_pool) = pools
    bq_sb, bk_sb, bv_rep, ones_sb = consts
    q_in_c, k_in_c, v_in_c, Wq_c, Wk_c, Wv_c, out_r = tensors

    # kT_sb[:, i] = i-th rank's key half, feature-major [DOT, 1024]
    kT_sb = kt_pool.tile([P, 2, DOT, LKS], DT, tag="kT", name="kT_sb")
    # v_sb[:, i] = i-th rank's value half, [8 lk-chunks x 1024 d] flat
    v_sb = v_pool.tile([P, 2, (LKT // 2) * D], DT, tag="v", name="v_sb")

    # ---- DMA issue ----------------------------------------------------
    Wk_sb = w_pool.tile([P, KCH, D], DT, tag="W", name="Wk_sb")
    nc.sync.dma_start(Wk_sb[:], Wk_c[:])
    kin = []
    for n in range(NBK):
        t = kin_pool.tile([P, KCH, PBLK], DT, tag="kin", name=f"kin{n}")
        nc.sync.dma_start(t[:], k_in_c[n])
        kin.append(t)
    Wv_sb = w_pool.tile([P, KCH, D], DT, tag="W", name="Wv_sb")
    nc.sync.dma_start(Wv_sb[:], Wv_c[:])
    vin = []
    for n in range(NBK):
        t = vin_pool.tile([P, KCH, PBLK], DT, tag="vin", name=f"vin{n}")
        nc.sync.dma_start(t[:], v_in_c[n])
        vin.append(t)
    Wq_sb = w_pool.tile([P, KCH, D], DT, tag="W", name="Wq_sb")
    nc.scalar.dma_start(Wq_sb[:], Wq_c[:])
    qin = [None] * NBQ
    for n in range(2):
        qin[n] = qin_pool.tile([P, KCH, ABLK], DT, tag="qin", name=f"qin{n}")
        nc.scalar.dma_start(qin[n][:], q_in_c[n])

    bounce_k = dram.tile([P, DOT, LKS], DT, tag="bnk", name="bounce_k")
    gath_k = dram.tile([2, P, DOT, LKS], DT, tag="gak", name="gath_k")
    bounce_v = dram.tile([P, LKT, PBLK], DT, tag="bnv", name="bounce_v")
    gath_v = dram.tile([2, P, LKT, PBLK], DT, tag="gav", name="gath_v")

    # ---- phase A: kT proj of own half -> bounce -> AllGather ----------
    for n in range(NBK):
        stg = stage_pool.tile([P, DOT, PBLK], DT, tag="stage", name=f"stgk{n}")
        for m in range(DOT):
            ps = psum_mm.tile([P, PBLK], F32, tag="mm", name="ps_k")
            for k in range(KCH):
                nc.tensor.matmul(
                    ps[:],
                    Wk_sb[:, k, m * P : (m + 1) * P],
                    kin[n][:, k],
                    start=(k == 0),
                    stop=(k == KCH - 1),
                )
            nc.vector.tensor_scalar_add(stg[:, m], ps[:], bk_sb[:, m : m + 1])
        nc.sync.dma_start(bounce_k[:, :, n * PBLK : (n + 1) * PBLK], stg[:])
    nc.gpsimd.collective_compute(
        "AllGather",
        mybir.AluOpType.bypass,
        replica_groups=groups,
        ins=[bounce_k.opt()],
        outs=[gath_k.opt()],
    )
    for i in range(2):
        nc.sync.dma_start(kT_sb[:, i], gath_k[i])

    # ---- phase B: v proj of own half -> bounce -> AllGather -----------
    for n in range(NBK):
        stg = stage_pool.tile([P, DOT, PBLK], DT, tag="stage", name=f"stgv{n}")
        for t in range(PBLK // P):
            for dh in range(D // PBLK):
                ps = psum_mm.tile([P, PBLK], F32, tag="mm", name="ps_v")
                for k in range(KCH):
                    nc.tensor.matmul(
                        ps[:],
                        vin[n][:, k, t * P : (t + 1) * P],
                        Wv_sb[:, k, dh * PBLK : (dh + 1) * PBLK],
                        start=(k == 0),
                        stop=(k == KCH - 1),
                    )
                nc.vector.tensor_add(
                    stg[:, 2 * t + dh],
                    ps[:],
                    bv_rep[:, dh * PBLK : (dh + 1) * PBLK],
                )
        nc.scalar.dma_start(bounce_v[:, n * DOT : (n + 1) * DOT], stg[:])
    nc.gpsimd.collective_compute(
        "AllGather",
        mybir.AluOpType.bypass,
        replica_groups=groups,
        ins=[bounce_v.opt()],
        outs=[gath_v.opt()],
    )
    for i in range(2):
        nc.sync.dma_start(
            v_sb[:, i].rearrange("p (c j) -> p c j", j=PBLK), gath_v[i]
        )

    # ---- phase C: q projection for all 4 blocks -----------------------
    qtb = [None] * NBQ
    for blk in range(NBQ):
        if blk + 2 < NBQ:
            qin[blk + 2] = qin_pool.tile(
                [P, KCH, ABLK], DT, tag="qin", name=f"qin{blk + 2}"
            )
            nc.scalar.dma_start(qin[blk + 2][:], q_in_c[blk + 2])
        qtb[blk] = qtb_pool.tile([P, KCH, ABLK], DT, tag="qtb", name=f"qtb{blk}")
        for m in range(DOT):
            ps = psum_st.tile([P, ABLK], F32, tag="st", name="ps_qp")
            for k in range(KCH):
                nc.tensor.matmul(
                    ps[:],
                    Wq_sb[:, k, m * P : (m + 1) * P],
                    qin[blk][:, k],
                    start=(k == 0),
                    stop=(k == KCH - 1),
                )
            nc.vector.tensor_scalar_add(
                qtb[blk][:, m], ps[:], bq_sb[:, m : m + 1]
            )

    # ---- phase D: attention ------------------------------------------
    for blk in range(NBQ):
        o_ps = [
            [
                psum_mm.tile([P, PBLK], F32, tag="mm", name=f"o_ps_{t}_{dh}")
                for dh in range(D // PBLK)
            ]
            for t in range(ABLK // P)
        ]
        r_ps = [
            psum_r.tile([P, 2], F32, tag="r", name=f"r_ps_{t}")
            for t in range(ABLK // P)
        ]

        def scores(ci, _qtb=qtb[blk]):
            half, c = divmod(ci, LKT // 2)
            st = psum_st.tile([P, ABLK], F32, tag="st", name="st")
            for o in range(KCH):
                nc.tensor.matmul(
                    st[:],
                    kT_sb[:, half, o, c * P : (c + 1) * P],
                    _qtb[:, o],
                    start=(o == 0),
                    stop=(o == KCH - 1),
                )
            return st

        # All scores (+exp into SBUF) first, then all pv: exp latency is
        # then never on the PE critical path.
        pts = []
        for ci in range(LKT):
            st = scores(ci)
            pt = pt_pool.tile([P, ABLK], DT, tag="pt", name="pt")
            nc.scalar.activation(
                pt[:], st[:], mybir.ActivationFunctionType.Exp, scale=SCALE
            )
            pts.append(pt)
        for ci in range(LKT):
            half, c = divmod(ci, LKT // 2)
            pt = pts[ci]
            for t in range(ABLK // P):
                pt_t = pt[:, t * P : (t + 1) * P]
                for dh in range(D // PBLK):
                    nc.tensor.matmul(
                        o_ps[t][dh][:],
                        pt_t,
                        v_sb[:, half, c * D + dh * PBLK : c * D + (dh + 1) * PBLK],
                        start=(ci == 0),
                        stop=(ci == LKT - 1),
                    )
                nc.tensor.matmul(
                    r_ps[t][:],
                    pt_t,
                    ones_sb[:],
                    start=(ci == 0),
                    stop=(ci == LKT - 1),
                )

        osb = osb_pool.tile([P, ABLK // P, D], F32, tag="osb", name="osb")
        for t in range(ABLK // P):
            rsb = rsb_pool.tile([P, 1], F32, tag="rsb", name="rsb")
            nc.vector.reciprocal(rsb[:], r_ps[t][:, 0:1])
            for dh in range(D // PBLK):
                nc.vector.tensor_scalar_mul(
                    osb[:, t, dh * PBLK : (dh + 1) * PBLK], o_ps[t][dh][:], rsb[:]
                )
        nc.scalar.dma_start(out_r[blk], osb[:])


_program = None


def _get_program():
    global _program
    if _program is None:
        _program = build_program()
    return _program


def _chunk_lin(x, blk):
    """[L, Din] -> [L//blk, P, KCH, blk] with [n,p,o,l] = x[n*blk+l, o*P+p]."""
    L = x.shape[0]
    return np.ascontiguousarray(
        x.reshape(L // blk, blk, KCH, P).transpose(0, 3, 2, 1)
    )


def _chunk_w(w):
    """[Din, Dout] -> [P, KCH, Dout] with [p,o,n] = w[o*P+p, n]."""
    return np.ascontiguousarray(w.reshape(KCH, P, D).transpose(1, 0, 2))


def _make_in_maps(query_input, key_input, value_input, Wq, bq, Wk, bk, Wv, bv):
    f32 = np.float32
    Wq_h = _chunk_w(np.asarray(Wq, HOST_DT))
    Wk_h = _chunk_w(np.asarray(Wk, HOST_DT))
    Wv_h = _chunk_w(np.asarray(Wv, HOST_DT))
    bq_h = np.asarray(bq, f32)
    bk_h = np.asarray(bk, f32)
    bv_h = np.asarray(bv, f32)
    in_maps = []
    for c in range(NCORES):
        b, h = divmod(c, 2)
        k_c = _chunk_lin(
            np.asarray(key_input[b, h * LKS : (h + 1) * LKS, :], HOST_DT), PBLK
        )
        v_c = _chunk_lin(
            np.asarray(value_input[b, h * LKS : (h + 1) * LKS, :], HOST_DT), PBLK
        )
        q_c = _chunk_lin(
            np.asarray(query_input[b, h * LQS : (h + 1) * LQS, :], HOST_DT), ABLK
        )
        in_maps.append(
            {
                "q_in_c": q_c,
                "k_in_c": k_c,
                "v_in_c": v_c,
                "Wq_c": Wq_h,
                "Wk_c": Wk_h,
                "Wv_c": Wv_h,
                "bq": bq_h,
                "bk": bk_h,
                "bv": bv_h,
            }
        )
    return in_maps


def run(in_maps, **kwargs):
    nc = _get_program()
    return run_bass_kernel_spmd(nc, in_maps, core_ids=list(range(NCORES)), **kwargs)


def kernel(query_input, key_input, value_input, Wq, bq, Wk, bk, Wv, bv):
    in_maps = _make_in_maps(
        query_input, key_input, value_input, Wq, bq, Wk, bk, Wv, bv
    )
    res = run(in_maps)
    out = np.empty((B, LQ, D), np.float32)
    for c in range(NCORES):
        b, h = divmod(c, 2)
        out[b, h * LQS : (h + 1) * LQS, :] = res.results[c]["out"]
    return out
